# revision 15
# baseline (speedup 1.0000x reference)
"""Hamilton-Adams demosaic kernel for Trainium2 (8 NeuronCores, data-parallel over batch).

Input:  x [8, 4, 768, 768] f32  (Bayer quarter-res planes: P00=R, P01=Gr, P10=Gb, P11=B)
Output: [8, 3, 1536, 1536] f32  (R, G, B full-res)

v2: phase-domain fp16 compute with the heavy vertical/diagonal accumulations moved to
the tensor engine (band matrices including row-shift identities), merged two-segment
DVE ops via custom strided access patterns, and scalar-engine PSUM drains with fused
abs/scale.  Interior strips need no SBUF->SBUF shift DMAs at all: row shifts ride in
the band matmuls, and halo columns are rewritten (mosaic-replication -> zero) between
the green and chroma passes.

Strips: 7 strips of 128 quarter-rows advancing by 124; lanes 2..125 shipped.
Plane quad tile PQ = [128, 4*770] (p00|p01|p10|p11), green quad GQ = [128, 2*770].
"""
import sys
sys.path.insert(0, '/opt/trn_rl_repo')

from contextlib import ExitStack

import numpy as np

import concourse.bass as bass
import concourse.bacc as bacc
import concourse.tile as tile
from concourse import mybir
from concourse.ap import AP
from bass_rust import VecI64Pair
from concourse.bass_utils import run_bass_kernel_spmd

F = mybir.dt.float32
H16 = mybir.dt.float16
U16 = mybir.dt.uint16
AL = mybir.AluOpType
AF = mybir.ActivationFunctionType

H = 768
SW = 770          # per-plane segment width (1-col halo each side)
NCORES = 8
STRIPS = [0, 124, 248, 372, 496, 620, 644]

BAND_ORDER = ["BA", "BB", "BC3", "BC5", "BA2", "BC5b", "BE", "BF", "BI",
              "ID", "IDu", "IDd", "IDn", "NH", "NHu", "NHd"]


def _bands_np():
    """lhsT band matrices W[k, m] = weight of input row k toward output row m."""
    E = lambda k=0: np.eye(128, k=k, dtype=np.float32)
    B = {
        "BA": 0.5 * E(1) + 0.5 * E(0),                   # 0.5*(in[m-1]+in[m])
        "BB": -0.25 * E(1) + 0.5 * E(0) - 0.25 * E(-1),  # -0.25*c3
        "BC3": 0.5 * E(1) - 1.0 * E(0) + 0.5 * E(-1),    # c3/2
        "BC5": 0.5 * E(1) - 0.5 * E(0),                  # (in[m-1]-in[m])/2
        "BA2": 0.5 * E(0) + 0.5 * E(-1),                 # 0.5*(in[m]+in[m+1])
        "BC5b": 0.5 * E(0) - 0.5 * E(-1),                # (in[m]-in[m+1])/2
        "BE": -0.25 * E(0) - 0.25 * E(-1),               # -0.25*(in[m]+in[m+1])
        "BF": -0.25 * E(1) - 0.25 * E(0),                # -0.25*(in[m-1]+in[m])
        "BI": 0.5 * E(0),                                # 0.5*in[m]
        "ID": E(0),                                      # in[m]
        "IDu": E(-1),                                    # in[m+1]
        "IDd": E(1),                                     # in[m-1]
        "IDn": -E(0),                                    # -in[m]
        "NH": -0.5 * E(0),
        "NHu": -0.5 * E(-1),
        "NHd": -0.5 * E(1),
    }
    return np.concatenate([B[n] for n in BAND_ORDER], axis=1).astype(np.float16)


def sv(t, cols, inner=768, step=1):
    """Multi-segment strided view of tile t: one AP with dims
    [partition, len(cols) segments, inner] where cols must be affine."""
    ap0 = t[:]
    pst = ap0.ap[0][0]
    stride = (cols[1] - cols[0]) if len(cols) > 1 else 1
    for i in range(2, len(cols)):
        assert cols[i] - cols[i - 1] == stride
    return AP(tensor=ap0.tensor, offset=cols[0],
              ap=VecI64Pair([[pst, 128], [stride, len(cols)], [step, inner]]))


class S:
    def __init__(self, nc, pools, r0, first, last):
        self.nc = nc
        self.r0, self.first, self.last = r0, first, last
        pl, gr, tmp, out, psp, ez = pools
        self.pq = pl.tile([128, 4 * SW], H16, tag="pq", name="pq")
        self.gq = gr.tile([128, 2 * SW], H16, tag="gq", name="gq")
        self.out3 = out.tile([128, 9216], H16, tag="out3", name="out3")
        self._tmp, self._psp, self._ez = tmp, psp, ez

    def tmp(self, n=1536):
        bufs = 4 if n != 1536 else None
        return self._tmp.tile([128, n], H16, tag=f"tmp{n}", name=f"tmp{n}", bufs=bufs)

    def msk(self):
        return self._tmp.tile([128, 1536], U16, tag="msk", name="msk", bufs=3)

    def ez(self):  # edge-strip helper tiles [128, SW]
        return self._ez.tile([128, SW], H16, tag="ez", name="ez")

    def ps(self):
        return self._psp.tile([128, 1024], F, tag="ps", name="ps", bufs=4)

    def psc(self):
        return self._psp.tile([128, 1024], F, tag="ps", name="ps", bufs=4)

    # --- plane views (main convention: plane[j] at col j+1 of its segment) ---
    def pqv(self, segs, d):
        return sv(self.pq, [sg * SW + 1 + d for sg in segs])

    def p(self, seg, d=0):  # single plane interior view
        return self.pq[:, seg * SW + 1 + d: seg * SW + 769 + d]

    def g(self, seg, d=0):
        return self.gq[:, seg * SW + 1 + d: seg * SW + 769 + d]

    # --- output views: channel c in {0:R,1:G,2:B}, phase (a,b) ---
    def osl(self, c, a, b):
        base = c * 3072 + a * 1536 + b
        return self.out3[:, base: base + 1536: 2]

    def osl2(self, spec0, spec1):
        b0 = spec0[0] * 3072 + spec0[1] * 1536 + spec0[2]
        b1 = spec1[0] * 3072 + spec1[1] * 1536 + spec1[2]
        return sv(self.out3, [b0, b1], inner=768, step=2)


def _mm(nc, ps, band, rhs, start, stop):
    nc.tensor.matmul(ps[:, 0:512], band, rhs[:, 0:512], start=start, stop=stop,
                     skip_group_check=True)
    nc.tensor.matmul(ps[:, 512:768], band, rhs[:, 512:768], start=start, stop=stop,
                     skip_group_check=True)


def _accum(nc, ps, terms, start=True, stop=True):
    """terms: list of (band, rhs[128 x 768] view); accumulate into ps cols 0:768."""
    n = len(terms)
    for i, (band, rhs) in enumerate(terms):
        _mm(nc, ps, band, rhs, start and i == 0, stop and i == n - 1)


def _load_strip(nc, s, x):
    base = s.r0 - 2
    pq = s.pq
    clo, chi = max(base, 0), min(base + 128, H)
    src = x[:, clo:chi, :].rearrange("s r c -> r s c")
    dst = pq[clo - base: chi - base, :].rearrange("p (sg c) -> p sg c", sg=4)[:, :, 1:769]
    nc.gpsimd.dma_start(dst, src)

    def fillp(seg, lane, c, row):
        nc.gpsimd.dma_start(pq[lane: lane + 1, seg * SW + 1: seg * SW + 769],
                            x[c, row: row + 1, :])

    if s.first:
        for seg, c in ((0, 0), (1, 1), (2, 0), (3, 1)):
            fillp(seg, 0, c, 0); fillp(seg, 1, c, 0)
    if s.last:
        for seg, c in ((0, 2), (1, 3), (2, 2), (3, 3)):
            fillp(seg, 126, c, 767); fillp(seg, 127, c, 767)


def _build_strip(nc, s, x, out_r, B):
    r0, first, last = s.r0, s.first, s.last
    base = r0 - 2
    pq, gq, out3 = s.pq, s.gq, s.out3

    # mosaic-replication halo columns (green pass wants these)
    cc = nc.vector.tensor_copy
    cc(pq[:, 0:1], pq[:, 1:2])                       # p00[-1] = p00[0]
    cc(pq[:, SW: SW + 1], pq[:, 1:2])                # p01[-1] = p00[0]
    cc(pq[:, 3 * SW: 3 * SW + 1], pq[:, 2 * SW + 1: 2 * SW + 2])   # p11[-1] = p10[0]
    cc(pq[:, SW - 1: SW], pq[:, SW + 768: SW + 769])               # p00[768] = p01[767]
    cc(pq[:, 3 * SW - 1: 3 * SW], pq[:, 4 * SW - 2: 4 * SW - 1])   # p10[768] = p11[767]
    cc(pq[:, 4 * SW - 1: 4 * SW], pq[:, 4 * SW - 2: 4 * SW - 1])   # p11[768] = p11[767]

    # ---------------- green (both phases; merged seg order [phase00, phase11]) ----
    # phase00: cp=p00(seg0) hp=p01(seg1) ho=(-1,0) vp=p10(seg2) bands BA/BC5
    # phase11: cp=p11(seg3) hp=p10(seg2) ho=(0,+1) vp=p01(seg1) bands BA2/BC5b
    psA0, psC30, psC50 = s.ps(), s.ps(), s.ps()
    _accum(nc, psA0, [(B("BA"), s.p(2)), (B("BB"), s.p(0))])
    _accum(nc, psC30, [(B("BC3"), s.p(0))])
    _accum(nc, psC50, [(B("BC5"), s.p(2))])
    psA1, psC31, psC51 = s.ps(), s.ps(), s.ps()
    _accum(nc, psA1, [(B("BA2"), s.p(1)), (B("BB"), s.p(3))])
    _accum(nc, psC31, [(B("BC3"), s.p(3))])
    _accum(nc, psC51, [(B("BC5b"), s.p(1))])

    ahm = s.tmp(); q3m = s.tmp(); q5m = s.tmp()
    nc.vector.tensor_scalar(ahm[:, 0:768], psA0[:, 0:768], 1.0, None, AL.mult)
    nc.vector.tensor_scalar(ahm[:, 768:1536], psA1[:, 0:768], 1.0, None, AL.mult)
    nc.scalar.activation(q3m[:, 0:768], psC30[:, 0:768], AF.Abs)
    nc.scalar.activation(q3m[:, 768:1536], psC31[:, 0:768], AF.Abs)
    nc.scalar.activation(q5m[:, 0:768], psC50[:, 0:768], AF.Abs)
    nc.scalar.activation(q5m[:, 768:1536], psC51[:, 0:768], AF.Abs)

    cpv = lambda d: s.pqv((0, 3), d)
    hpa = sv(pq, [SW + 0, 2 * SW + 1])       # [p01(-1), p10(0)]
    hpb = sv(pq, [SW + 1, 2 * SW + 2])       # [p01(0),  p10(+1)]
    sh = s.tmp();  nc.vector.tensor_tensor(sh[:], cpv(-1), cpv(+1), AL.add)
    ch2 = s.tmp(); nc.vector.scalar_tensor_tensor(ch2[:], sh[:], 0.5, cpv(0), AL.mult, AL.subtract)
    c0s = s.tmp(); nc.vector.tensor_tensor(c0s[:], hpa, hpb, AL.add)
    c4d = s.tmp(); nc.vector.tensor_tensor(c4d[:], hpa, hpb, AL.subtract)
    b2 = s.tmp();  nc.vector.tensor_tensor(b2[:], c0s[:], ch2[:], AL.subtract)
    q2 = s.tmp();  nc.scalar.activation(q2[:], ch2[:], AF.Abs)
    q4 = s.tmp();  nc.scalar.activation(q4[:], c4d[:], AF.Abs, scale=0.5)
    clh = s.tmp(); nc.vector.tensor_tensor(clh[:], q4[:], q2[:], AL.add)
    clv = s.tmp(); nc.vector.tensor_tensor(clv[:], q5m[:], q3m[:], AL.add)
    d = s.tmp();   nc.vector.tensor_tensor(d[:], clh[:], clv[:], AL.subtract)
    m = s.msk();   nc.vector.tensor_scalar(m[:], d[:], 0.0, None, AL.is_gt)
    gint = sv(gq, [1, SW + 1])
    nc.vector.tensor_scalar(gint, b2[:], 0.5, None, AL.mult)
    nc.vector.copy_predicated(gint, m[:], ahm[:])

    # green halo cols + edge rows
    cc(gq[:, SW - 1: SW], pq[:, SW + 768: SW + 769])    # g00[768] = p01[767]
    cc(gq[:, SW: SW + 1], pq[:, 2 * SW + 1: 2 * SW + 2])  # g11[-1] = p10[0]
    if first:  # green at virtual row -1 on (1,1) sites = p01 row 0 (lane 1)
        nc.gpsimd.dma_start(gq[1:2, SW: 2 * SW], pq[1:2, SW: 2 * SW])
    if last:   # green at virtual row 768 on (0,0) sites = p10 row 767 (lane 126)
        nc.gpsimd.dma_start(gq[126:127, 0:SW], pq[126:127, 2 * SW: 3 * SW])

    # ---------------- rewrite halos for the chroma pass (masked-plane replication = 0)
    nc.vector.memset(pq[:, SW - 1: SW], 0.0)        # p00[768] := 0
    nc.vector.memset(pq[:, 3 * SW: 3 * SW + 1], 0.0)  # p11[-1] := 0
    if last:   # p00 rows beyond image bottom := 0 for R11 chroma
        nc.gpsimd.dma_start(pq[126:128, 0:SW], s.zt[0:2, :])
    if first:  # p11 rows above image top := 0 for B00 chroma
        nc.gpsimd.dma_start(pq[0:2, 3 * SW: 4 * SW], s.zt[0:2, :])

    # ---------------- green channel output ----------------
    nc.vector.tensor_copy(s.osl2((1, 0, 0), (1, 1, 1)), sv(gq, [1, SW + 1]))
    nc.scalar.copy(s.osl2((1, 0, 1), (1, 1, 0)), sv(pq, [SW + 1, 2 * SW + 1]))
    nc.scalar.copy(s.osl2((0, 0, 0), (2, 1, 1)), sv(pq, [1, 3 * SW + 1]))

    # ---------------- R01 + B10 (merged horizontal interp) ----------------
    # R01 = 0.5*(p00[j]+p00[j+1]) - 0.25*(g00[j]+g00[j+1]) + 0.5*p01
    # B10 = 0.5*(p11[j-1]+p11[j]) - 0.25*(g11[j-1]+g11[j]) + 0.5*p10
    t1 = s.tmp(); nc.vector.tensor_tensor(t1[:], sv(pq, [1, 3 * SW]), sv(pq, [2, 3 * SW + 1]), AL.add)
    t2 = s.tmp(); nc.vector.tensor_tensor(t2[:], sv(gq, [1, SW]), sv(gq, [2, SW + 1]), AL.add)
    u = s.tmp();  nc.vector.scalar_tensor_tensor(u[:], t2[:], -0.5, t1[:], AL.mult, AL.add)
    v = s.tmp();  nc.vector.tensor_tensor(v[:], u[:], sv(pq, [SW + 1, 2 * SW + 1]), AL.add)
    nc.scalar.mul(s.osl2((0, 0, 1), (2, 1, 0)), v[:], 0.5)

    # ---------------- R10 / B01 (vertical interp via bands) ----------------
    if not last:
        psD = s.ps()
        _accum(nc, psD, [(B("BA2"), s.p(0)), (B("BE"), s.g(0)), (B("BI"), s.p(2))])
        nc.scalar.copy(s.osl(0, 1, 0), psD[:, 0:768])
    if not first:
        psE = s.ps()
        _accum(nc, psE, [(B("BA"), s.p(3)), (B("BF"), s.g(1)), (B("BI"), s.p(1))])
        nc.scalar.copy(s.osl(2, 0, 1), psE[:, 0:768])

    # legacy single-phase paths at the image top/bottom (zero row replication)
    if last:
        p00uz = s.ez(); g00u = s.ez()
        nc.vector.memset(p00uz[:], 0.0)
        nc.gpsimd.dma_start(p00uz[0:125, 0:769], pq[1:126, 0:769])
        nc.gpsimd.dma_start(g00u[0:127, :], gq[1:128, 0:SW])
        lt1 = s.tmp(768); nc.vector.tensor_tensor(lt1[:], s.p(0), p00uz[:, 1:769], AL.add)
        lt2 = s.tmp(768); nc.vector.tensor_tensor(lt2[:], s.g(0), g00u[:, 1:769], AL.add)
        lu = s.tmp(768);  nc.vector.scalar_tensor_tensor(lu[:], lt2[:], -0.5, lt1[:], AL.mult, AL.add)
        lv = s.tmp(768);  nc.vector.tensor_tensor(lv[:], lu[:], s.p(2), AL.add)
        nc.scalar.mul(s.osl(0, 1, 0), lv[:], 0.5)
    if first:
        p11dz = s.ez(); g11d = s.ez()
        nc.vector.memset(p11dz[:], 0.0)
        nc.gpsimd.dma_start(p11dz[3:128, 1:770], pq[2:127, 3 * SW + 1: 4 * SW])
        nc.gpsimd.dma_start(g11d[1:128, :], gq[0:127, SW: 2 * SW])
        lt1 = s.tmp(768); nc.vector.tensor_tensor(lt1[:], p11dz[:, 1:769], s.p(3), AL.add)
        lt2 = s.tmp(768); nc.vector.tensor_tensor(lt2[:], g11d[:, 1:769], s.g(1), AL.add)
        lu = s.tmp(768);  nc.vector.scalar_tensor_tensor(lu[:], lt2[:], -0.5, lt1[:], AL.mult, AL.add)
        lv = s.tmp(768);  nc.vector.tensor_tensor(lv[:], lu[:], s.p(1), AL.add)
        nc.scalar.mul(s.osl(2, 0, 1), lv[:], 0.5)

    # ---------------- R11 + B00 (diagonal chroma, tensor-engine chains) ----------
    # R11: rows via E(-1) ("u" = row+1); B00: rows via E(+1) ("d" = row-1).
    qpm = s.tmp(); qnm = s.tmp(); qdmm = s.tmp(); qdnm = s.tmp(); cnhm = s.tmp()
    for phi, (xseg, gseg, nseg, UP, osl_ab) in enumerate((
            (0, 0, 1, True, (0, 1, 1)),     # R11: x=p00, gf=g00, near=g11
            (3, 1, 0, False, (2, 0, 0)))):  # B00: x=p11, gf=g11, near=g00
        sl = slice(phi * 768, phi * 768 + 768)
        shf = "IDu" if UP else "IDd"
        nh_sh = "NHu" if UP else "NHd"
        da, db = (0, +1) if UP else (0, -1)  # col offsets: da=unshifted row, db=shifted row
        psP = s.psc()
        _accum(nc, psP, [(B("NH"), s.g(gseg, da)), (B(nh_sh), s.g(gseg, db)),
                         (B("ID"), s.g(nseg))], stop=False)
        nc.scalar.activation(qpm[:, sl], psP[:, 0:768], AF.Abs)
        _accum(nc, psP, [(B("ID"), s.p(xseg, da)), (B(shf), s.p(xseg, db))], start=False)
        nc.scalar.mul(s.osl(*osl_ab), psP[:, 0:768], 0.5)
        psN = s.psc()
        _accum(nc, psN, [(B("NH"), s.g(gseg, db)), (B(nh_sh), s.g(gseg, da)),
                         (B("ID"), s.g(nseg))], stop=False)
        nc.scalar.activation(qnm[:, sl], psN[:, 0:768], AF.Abs)
        _accum(nc, psN, [(B("ID"), s.p(xseg, db)), (B(shf), s.p(xseg, da))], start=False)
        nc.scalar.mul(cnhm[:, sl], psN[:, 0:768], 0.5)
        psDM = s.psc()
        _accum(nc, psDM, [(B(shf), s.p(xseg, db)), (B("IDn"), s.p(xseg, da))])
        nc.scalar.activation(qdmm[:, sl], psDM[:, 0:768], AF.Abs, scale=0.5)
        psDN = s.psc()
        _accum(nc, psDN, [(B(shf), s.p(xseg, da)), (B("IDn"), s.p(xseg, db))])
        nc.scalar.activation(qdnm[:, sl], psDN[:, 0:768], AF.Abs, scale=0.5)

    clp = s.tmp(); nc.vector.tensor_tensor(clp[:], qdmm[:], qpm[:], AL.add)
    cln = s.tmp(); nc.vector.tensor_tensor(cln[:], qdnm[:], qnm[:], AL.add)
    dr = s.tmp();  nc.vector.tensor_tensor(dr[:], clp[:], cln[:], AL.subtract)
    mr = s.msk();  nc.vector.tensor_scalar(mr[:], dr[:], 0.0, None, AL.is_gt)
    nc.vector.copy_predicated(s.osl2((0, 1, 1), (2, 0, 0)), mr[:], cnhm[:])

    # ---------------- store ----------------
    if last:
        p0, pn, row0 = 102, 24, 744
    else:
        p0, pn, row0 = 2, 124, r0
    src3 = out3[p0: p0 + pn, :].rearrange("p (c w) -> p c w", c=3)
    nc.gpsimd.dma_start(out_r[row0: row0 + pn, :, :], src3)


def build_nc():
    nc = bacc.Bacc("TRN2", target_bir_lowering=False, debug=False, num_devices=NCORES)
    x_in = nc.declare_dram_parameter("x", [4, H, H], F, isOutput=False)
    bands_in = nc.declare_dram_parameter("bands", [128, len(BAND_ORDER) * 128], H16, isOutput=False)
    out = nc.declare_dram_parameter("out", [3, 2 * H, 2 * H], F, isOutput=True)
    out_r = out[:].rearrange("c (r two) w -> r c (two w)", two=2)

    with tile.TileContext(nc) as tc, ExitStack() as ctx:
        cst = ctx.enter_context(tc.tile_pool(name="consts", bufs=1))
        pl = ctx.enter_context(tc.tile_pool(name="planes", bufs=2))
        gr = ctx.enter_context(tc.tile_pool(name="greens", bufs=3))
        tmp = ctx.enter_context(tc.tile_pool(name="temps", bufs=32))
        outp = ctx.enter_context(tc.tile_pool(name="outs", bufs=2))
        psp = ctx.enter_context(tc.tile_pool(name="ps", bufs=2, space=bass.MemorySpace.PSUM))
        ez = ctx.enter_context(tc.tile_pool(name="ez", bufs=2))
        bands = cst.tile([128, len(BAND_ORDER) * 128], H16, tag="bands", name="bands")
        nc.sync.dma_start(bands[:], bands_in[:])
        zt = cst.tile([128, SW], H16, tag="zt", name="zt")
        nc.vector.memset(zt[:], 0.0)
        B = lambda n: bands[:, BAND_ORDER.index(n) * 128: BAND_ORDER.index(n) * 128 + 128]
        order = [0, len(STRIPS) - 1] + list(range(1, len(STRIPS) - 1))
        strips = []
        for si in order:
            s = S(nc, (pl, gr, tmp, outp, psp, ez), STRIPS[si], si == 0,
                  si == len(STRIPS) - 1)
            s.zt = zt
            strips.append(s)
        for i, s in enumerate(strips):
            _load_strip(nc, s, x_in[:])
            _build_strip(nc, s, x_in[:], out_r, B)
    nc.compile()
    return nc


_NC_CACHE = None


def kernel(x: np.ndarray) -> np.ndarray:
    global _NC_CACHE
    if _NC_CACHE is None:
        _NC_CACHE = build_nc()
    x = np.ascontiguousarray(x, dtype=np.float32)
    bnp = _bands_np()
    in_maps = [{"x": x[i], "bands": bnp} for i in range(NCORES)]
    res = run_bass_kernel_spmd(_NC_CACHE, in_maps, list(range(NCORES)))
    return np.stack([res.results[i]["out"] for i in range(NCORES)], axis=0)


# revision 17
# speedup vs baseline: 1.0668x; 1.0668x over previous
"""Hamilton-Adams demosaic kernel for Trainium2 (8 NeuronCores, data-parallel over batch).

Input:  x [8, 4, 768, 768] f32  (Bayer quarter-res planes: P00=R, P01=Gr, P10=Gb, P11=B)
Output: [8, 3, 1536, 1536] f32  (R, G, B full-res)

v2: phase-domain fp16 compute with the heavy vertical/diagonal accumulations moved to
the tensor engine (band matrices including row-shift identities), merged two-segment
DVE ops via custom strided access patterns, and scalar-engine PSUM drains with fused
abs/scale.  Interior strips need no SBUF->SBUF shift DMAs at all: row shifts ride in
the band matmuls, and halo columns are rewritten (mosaic-replication -> zero) between
the green and chroma passes.

Strips: 7 strips of 128 quarter-rows advancing by 124; lanes 2..125 shipped.
Plane quad tile PQ = [128, 4*770] (p00|p01|p10|p11), green quad GQ = [128, 2*770].
"""
import sys
sys.path.insert(0, '/opt/trn_rl_repo')

from contextlib import ExitStack

import numpy as np

import concourse.bass as bass
import concourse.bacc as bacc
import concourse.tile as tile
from concourse import mybir
from concourse.ap import AP
from bass_rust import VecI64Pair
from concourse.bass_utils import run_bass_kernel_spmd

F = mybir.dt.float32
H16 = mybir.dt.float16
U16 = mybir.dt.uint16
AL = mybir.AluOpType
AF = mybir.ActivationFunctionType

H = 768
SW = 770          # per-plane segment width (1-col halo each side)
NCORES = 8
STRIPS = [0, 124, 248, 372, 496, 620, 644]
CHAN = ((0, 0, 1, True, (0, 1, 1)),     # R11: x=p00, gf=g00, near=g11
        (3, 1, 0, False, (2, 0, 0)))    # B00: x=p11, gf=g11, near=g00

BAND_ORDER = ["BA", "BB", "BC3", "BC5", "BA2", "BC5b", "BE", "BF", "BI",
              "ID", "IDu", "IDd", "IDn", "NH", "NHu", "NHd"]


def _bands_np():
    """lhsT band matrices W[k, m] = weight of input row k toward output row m."""
    E = lambda k=0: np.eye(128, k=k, dtype=np.float32)
    B = {
        "BA": 0.5 * E(1) + 0.5 * E(0),                   # 0.5*(in[m-1]+in[m])
        "BB": -0.25 * E(1) + 0.5 * E(0) - 0.25 * E(-1),  # -0.25*c3
        "BC3": 0.5 * E(1) - 1.0 * E(0) + 0.5 * E(-1),    # c3/2
        "BC5": 0.5 * E(1) - 0.5 * E(0),                  # (in[m-1]-in[m])/2
        "BA2": 0.5 * E(0) + 0.5 * E(-1),                 # 0.5*(in[m]+in[m+1])
        "BC5b": 0.5 * E(0) - 0.5 * E(-1),                # (in[m]-in[m+1])/2
        "BE": -0.25 * E(0) - 0.25 * E(-1),               # -0.25*(in[m]+in[m+1])
        "BF": -0.25 * E(1) - 0.25 * E(0),                # -0.25*(in[m-1]+in[m])
        "BI": 0.5 * E(0),                                # 0.5*in[m]
        "ID": E(0),                                      # in[m]
        "IDu": E(-1),                                    # in[m+1]
        "IDd": E(1),                                     # in[m-1]
        "IDn": -E(0),                                    # -in[m]
        "NH": -0.5 * E(0),
        "NHu": -0.5 * E(-1),
        "NHd": -0.5 * E(1),
    }
    return np.concatenate([B[n] for n in BAND_ORDER], axis=1).astype(np.float16)


def sv(t, cols, inner=768, step=1):
    """Multi-segment strided view of tile t: one AP with dims
    [partition, len(cols) segments, inner] where cols must be affine."""
    ap0 = t[:]
    pst = ap0.ap[0][0]
    stride = (cols[1] - cols[0]) if len(cols) > 1 else 1
    for i in range(2, len(cols)):
        assert cols[i] - cols[i - 1] == stride
    return AP(tensor=ap0.tensor, offset=cols[0],
              ap=VecI64Pair([[pst, 128], [stride, len(cols)], [step, inner]]))


class S:
    def __init__(self, nc, pools, r0, first, last):
        self.nc = nc
        self.r0, self.first, self.last = r0, first, last
        pl, gr, tmp, out, psp, ez = pools
        self.pq = pl.tile([128, 4 * SW], H16, tag="pq", name="pq")
        self.gq = gr.tile([128, 2 * SW], H16, tag="gq", name="gq")
        self.out3 = out.tile([128, 9216], H16, tag="out3", name="out3")
        self._tmp, self._psp, self._ez = tmp, psp, ez

    def tmp(self, n=1536):
        bufs = 4 if n != 1536 else None
        return self._tmp.tile([128, n], H16, tag=f"tmp{n}", name=f"tmp{n}", bufs=bufs)

    def msk(self):
        return self._tmp.tile([128, 1536], U16, tag="msk", name="msk", bufs=3)

    def ez(self):  # edge-strip helper tiles [128, SW]
        return self._ez.tile([128, SW], H16, tag="ez", name="ez")

    def ps(self):
        return self._psp.tile([128, 1024], F, tag="ps", name="ps", bufs=4)

    def psc(self):
        return self._psp.tile([128, 1024], F, tag="ps", name="ps", bufs=4)

    # --- plane views (main convention: plane[j] at col j+1 of its segment) ---
    def pqv(self, segs, d):
        return sv(self.pq, [sg * SW + 1 + d for sg in segs])

    def p(self, seg, d=0):  # single plane interior view
        return self.pq[:, seg * SW + 1 + d: seg * SW + 769 + d]

    def g(self, seg, d=0):
        return self.gq[:, seg * SW + 1 + d: seg * SW + 769 + d]

    # --- output views: channel c in {0:R,1:G,2:B}, phase (a,b) ---
    def osl(self, c, a, b):
        base = c * 3072 + a * 1536 + b
        return self.out3[:, base: base + 1536: 2]

    def osl2(self, spec0, spec1):
        b0 = spec0[0] * 3072 + spec0[1] * 1536 + spec0[2]
        b1 = spec1[0] * 3072 + spec1[1] * 1536 + spec1[2]
        return sv(self.out3, [b0, b1], inner=768, step=2)


def _mm(nc, ps, band, rhs, start, stop):
    nc.tensor.matmul(ps[:, 0:512], band, rhs[:, 0:512], start=start, stop=stop,
                     skip_group_check=True)
    nc.tensor.matmul(ps[:, 512:768], band, rhs[:, 512:768], start=start, stop=stop,
                     skip_group_check=True)


def _accum(nc, ps, terms, start=True, stop=True):
    """terms: list of (band, rhs[128 x 768] view); accumulate into ps cols 0:768."""
    n = len(terms)
    for i, (band, rhs) in enumerate(terms):
        _mm(nc, ps, band, rhs, start and i == 0, stop and i == n - 1)


def _load_strip(nc, s, x):
    base = s.r0 - 2
    pq = s.pq
    clo, chi = max(base, 0), min(base + 128, H)
    src = x[:, clo:chi, :].rearrange("s r c -> r s c")
    dst = pq[clo - base: chi - base, :].rearrange("p (sg c) -> p sg c", sg=4)[:, :, 1:769]
    nc.gpsimd.dma_start(dst, src)

    def fillp(seg, lane, c, row):
        nc.gpsimd.dma_start(pq[lane: lane + 1, seg * SW + 1: seg * SW + 769],
                            x[c, row: row + 1, :])

    if s.first:
        for seg, c in ((0, 0), (1, 1), (2, 0), (3, 1)):
            fillp(seg, 0, c, 0); fillp(seg, 1, c, 0)
    if s.last:
        for seg, c in ((0, 2), (1, 3), (2, 2), (3, 3)):
            fillp(seg, 126, c, 767); fillp(seg, 127, c, 767)


def _build_strip(nc, s, x, out_r, B):
    r0, first, last = s.r0, s.first, s.last
    base = r0 - 2
    pq, gq, out3 = s.pq, s.gq, s.out3

    # mosaic-replication halo columns (green pass wants these)
    cc = nc.vector.tensor_copy
    cc(pq[:, 0:1], pq[:, 1:2])                       # p00[-1] = p00[0]
    cc(pq[:, SW: SW + 1], pq[:, 1:2])                # p01[-1] = p00[0]
    cc(pq[:, 3 * SW: 3 * SW + 1], pq[:, 2 * SW + 1: 2 * SW + 2])   # p11[-1] = p10[0]
    cc(pq[:, SW - 1: SW], pq[:, SW + 768: SW + 769])               # p00[768] = p01[767]
    cc(pq[:, 3 * SW - 1: 3 * SW], pq[:, 4 * SW - 2: 4 * SW - 1])   # p10[768] = p11[767]
    cc(pq[:, 4 * SW - 1: 4 * SW], pq[:, 4 * SW - 2: 4 * SW - 1])   # p11[768] = p11[767]

    # ---------------- green (both phases; merged seg order [phase00, phase11]) ----
    # phase00: cp=p00(seg0) hp=p01(seg1) ho=(-1,0) vp=p10(seg2) bands BA/BC5
    # phase11: cp=p11(seg3) hp=p10(seg2) ho=(0,+1) vp=p01(seg1) bands BA2/BC5b
    psA0, psC30, psC50 = s.ps(), s.ps(), s.ps()
    _accum(nc, psA0, [(B("BA"), s.p(2)), (B("BB"), s.p(0))])
    _accum(nc, psC30, [(B("BC3"), s.p(0))])
    _accum(nc, psC50, [(B("BC5"), s.p(2))])
    psA1, psC31, psC51 = s.ps(), s.ps(), s.ps()
    _accum(nc, psA1, [(B("BA2"), s.p(1)), (B("BB"), s.p(3))])
    _accum(nc, psC31, [(B("BC3"), s.p(3))])
    _accum(nc, psC51, [(B("BC5b"), s.p(1))])

    ahm = s.tmp(); q3m = s.tmp(); q5m = s.tmp()
    nc.vector.tensor_scalar(ahm[:, 0:768], psA0[:, 0:768], 1.0, None, AL.mult)
    nc.vector.tensor_scalar(ahm[:, 768:1536], psA1[:, 0:768], 1.0, None, AL.mult)
    nc.scalar.activation(q3m[:, 0:768], psC30[:, 0:768], AF.Abs)
    nc.scalar.activation(q3m[:, 768:1536], psC31[:, 0:768], AF.Abs)
    nc.scalar.activation(q5m[:, 0:768], psC50[:, 0:768], AF.Abs)
    nc.scalar.activation(q5m[:, 768:1536], psC51[:, 0:768], AF.Abs)

    cpv = lambda d: s.pqv((0, 3), d)
    hpa = sv(pq, [SW + 0, 2 * SW + 1])       # [p01(-1), p10(0)]
    hpb = sv(pq, [SW + 1, 2 * SW + 2])       # [p01(0),  p10(+1)]
    sh = s.tmp();  nc.vector.tensor_tensor(sh[:], cpv(-1), cpv(+1), AL.add)
    ch2 = s.tmp(); nc.vector.scalar_tensor_tensor(ch2[:], sh[:], 0.5, cpv(0), AL.mult, AL.subtract)
    c0s = s.tmp(); nc.vector.tensor_tensor(c0s[:], hpa, hpb, AL.add)
    c4d = s.tmp(); nc.vector.tensor_tensor(c4d[:], hpa, hpb, AL.subtract)
    b2 = s.tmp();  nc.vector.tensor_tensor(b2[:], c0s[:], ch2[:], AL.subtract)
    q2 = s.tmp();  nc.scalar.activation(q2[:], ch2[:], AF.Abs)
    q4 = s.tmp();  nc.scalar.activation(q4[:], c4d[:], AF.Abs, scale=0.5)
    clh = s.tmp(); nc.vector.tensor_tensor(clh[:], q4[:], q2[:], AL.add)
    clv = s.tmp(); nc.vector.tensor_tensor(clv[:], q5m[:], q3m[:], AL.add)
    d = s.tmp();   nc.vector.tensor_tensor(d[:], clh[:], clv[:], AL.subtract)
    m = s.msk();   nc.vector.tensor_scalar(m[:], d[:], 0.0, None, AL.is_gt)
    gint = sv(gq, [1, SW + 1])
    nc.vector.tensor_scalar(gint, b2[:], 0.5, None, AL.mult)
    nc.vector.copy_predicated(gint, m[:], ahm[:])

    # green halo cols + edge rows
    cc(gq[:, SW - 1: SW], pq[:, SW + 768: SW + 769])    # g00[768] = p01[767]
    cc(gq[:, SW: SW + 1], pq[:, 2 * SW + 1: 2 * SW + 2])  # g11[-1] = p10[0]
    if first:  # green at virtual row -1 on (1,1) sites = p01 row 0 (lane 1)
        nc.gpsimd.dma_start(gq[1:2, SW: 2 * SW], pq[1:2, SW: 2 * SW])
    if last:   # green at virtual row 768 on (0,0) sites = p10 row 767 (lane 126)
        nc.gpsimd.dma_start(gq[126:127, 0:SW], pq[126:127, 2 * SW: 3 * SW])

    # ---------------- rewrite halos for the chroma pass (masked-plane replication = 0)
    nc.vector.memset(pq[:, SW - 1: SW], 0.0)        # p00[768] := 0
    nc.vector.memset(pq[:, 3 * SW: 3 * SW + 1], 0.0)  # p11[-1] := 0
    if last:   # p00 rows beyond image bottom := 0 for R11 chroma
        nc.gpsimd.dma_start(pq[126:128, 0:SW], s.zt[0:2, :])
    if first:  # p11 rows above image top := 0 for B00 chroma
        nc.gpsimd.dma_start(pq[0:2, 3 * SW: 4 * SW], s.zt[0:2, :])

    # ---------------- chroma diagonal differences (x-only, overlap with green) --
    qdmm = s.tmp(); qdnm = s.tmp()
    for phi, (xseg, gseg, nseg, UP, osl_ab) in enumerate(CHAN):
        sl = slice(phi * 768, phi * 768 + 768)
        shf = "IDu" if UP else "IDd"
        da, db = (0, +1) if UP else (0, -1)
        psDM = s.psc()
        _accum(nc, psDM, [(B(shf), s.p(xseg, db)), (B("IDn"), s.p(xseg, da))])
        nc.scalar.activation(qdmm[:, sl], psDM[:, 0:768], AF.Abs, scale=0.5)
        psDN = s.psc()
        _accum(nc, psDN, [(B(shf), s.p(xseg, da)), (B("IDn"), s.p(xseg, db))])
        nc.scalar.activation(qdnm[:, sl], psDN[:, 0:768], AF.Abs, scale=0.5)

    # ---------------- green channel output ----------------
    nc.vector.tensor_copy(s.osl2((1, 0, 0), (1, 1, 1)), sv(gq, [1, SW + 1]))
    nc.scalar.copy(s.osl2((1, 0, 1), (1, 1, 0)), sv(pq, [SW + 1, 2 * SW + 1]))
    nc.scalar.copy(s.osl2((0, 0, 0), (2, 1, 1)), sv(pq, [1, 3 * SW + 1]))

    # ---------------- R01 + B10 (merged horizontal interp) ----------------
    # R01 = 0.5*(p00[j]+p00[j+1]) - 0.25*(g00[j]+g00[j+1]) + 0.5*p01
    # B10 = 0.5*(p11[j-1]+p11[j]) - 0.25*(g11[j-1]+g11[j]) + 0.5*p10
    t1 = s.tmp(); nc.vector.tensor_tensor(t1[:], sv(pq, [1, 3 * SW]), sv(pq, [2, 3 * SW + 1]), AL.add)
    t2 = s.tmp(); nc.vector.tensor_tensor(t2[:], sv(gq, [1, SW]), sv(gq, [2, SW + 1]), AL.add)
    u = s.tmp();  nc.vector.scalar_tensor_tensor(u[:], t2[:], -0.5, t1[:], AL.mult, AL.add)
    v = s.tmp();  nc.vector.tensor_tensor(v[:], u[:], sv(pq, [SW + 1, 2 * SW + 1]), AL.add)
    nc.scalar.mul(s.osl2((0, 0, 1), (2, 1, 0)), v[:], 0.5)

    # ---------------- R10 / B01 (vertical interp via bands) ----------------
    if not last:
        psD = s.ps()
        _accum(nc, psD, [(B("BA2"), s.p(0)), (B("BE"), s.g(0)), (B("BI"), s.p(2))])
        nc.scalar.copy(s.osl(0, 1, 0), psD[:, 0:768])
    if not first:
        psE = s.ps()
        _accum(nc, psE, [(B("BA"), s.p(3)), (B("BF"), s.g(1)), (B("BI"), s.p(1))])
        nc.scalar.copy(s.osl(2, 0, 1), psE[:, 0:768])

    # legacy single-phase paths at the image top/bottom (zero row replication)
    if last:
        p00uz = s.ez(); g00u = s.ez()
        nc.vector.memset(p00uz[:], 0.0)
        nc.gpsimd.dma_start(p00uz[0:125, 0:769], pq[1:126, 0:769])
        nc.gpsimd.dma_start(g00u[0:127, :], gq[1:128, 0:SW])
        lt1 = s.tmp(768); nc.vector.tensor_tensor(lt1[:], s.p(0), p00uz[:, 1:769], AL.add)
        lt2 = s.tmp(768); nc.vector.tensor_tensor(lt2[:], s.g(0), g00u[:, 1:769], AL.add)
        lu = s.tmp(768);  nc.vector.scalar_tensor_tensor(lu[:], lt2[:], -0.5, lt1[:], AL.mult, AL.add)
        lv = s.tmp(768);  nc.vector.tensor_tensor(lv[:], lu[:], s.p(2), AL.add)
        nc.scalar.mul(s.osl(0, 1, 0), lv[:], 0.5)
    if first:
        p11dz = s.ez(); g11d = s.ez()
        nc.vector.memset(p11dz[:], 0.0)
        nc.gpsimd.dma_start(p11dz[3:128, 1:770], pq[2:127, 3 * SW + 1: 4 * SW])
        nc.gpsimd.dma_start(g11d[1:128, :], gq[0:127, SW: 2 * SW])
        lt1 = s.tmp(768); nc.vector.tensor_tensor(lt1[:], p11dz[:, 1:769], s.p(3), AL.add)
        lt2 = s.tmp(768); nc.vector.tensor_tensor(lt2[:], g11d[:, 1:769], s.g(1), AL.add)
        lu = s.tmp(768);  nc.vector.scalar_tensor_tensor(lu[:], lt2[:], -0.5, lt1[:], AL.mult, AL.add)
        lv = s.tmp(768);  nc.vector.tensor_tensor(lv[:], lu[:], s.p(1), AL.add)
        nc.scalar.mul(s.osl(2, 0, 1), lv[:], 0.5)

    # ---------------- R11 + B00 (diagonal chroma, tensor-engine chains) ----------
    # R11: rows via E(-1) ("u" = row+1); B00: rows via E(+1) ("d" = row-1).
    qpm = s.tmp(); qnm = s.tmp(); cnhm = s.tmp()
    for phi, (xseg, gseg, nseg, UP, osl_ab) in enumerate(CHAN):
        sl = slice(phi * 768, phi * 768 + 768)
        shf = "IDu" if UP else "IDd"
        nh_sh = "NHu" if UP else "NHd"
        da, db = (0, +1) if UP else (0, -1)  # col offsets: da=unshifted row, db=shifted row
        psP = s.psc()
        _accum(nc, psP, [(B("NH"), s.g(gseg, da)), (B(nh_sh), s.g(gseg, db)),
                         (B("ID"), s.g(nseg))], stop=False)
        nc.scalar.activation(qpm[:, sl], psP[:, 0:768], AF.Abs)
        _accum(nc, psP, [(B("ID"), s.p(xseg, da)), (B(shf), s.p(xseg, db))], start=False)
        nc.scalar.mul(s.osl(*osl_ab), psP[:, 0:768], 0.5)
        psN = s.psc()
        _accum(nc, psN, [(B("NH"), s.g(gseg, db)), (B(nh_sh), s.g(gseg, da)),
                         (B("ID"), s.g(nseg))], stop=False)
        nc.scalar.activation(qnm[:, sl], psN[:, 0:768], AF.Abs)
        _accum(nc, psN, [(B("ID"), s.p(xseg, db)), (B(shf), s.p(xseg, da))], start=False)
        nc.scalar.mul(cnhm[:, sl], psN[:, 0:768], 0.5)

    clp = s.tmp(); nc.vector.tensor_tensor(clp[:], qdmm[:], qpm[:], AL.add)
    cln = s.tmp(); nc.vector.tensor_tensor(cln[:], qdnm[:], qnm[:], AL.add)
    dr = s.tmp();  nc.vector.tensor_tensor(dr[:], clp[:], cln[:], AL.subtract)
    mr = s.msk();  nc.vector.tensor_scalar(mr[:], dr[:], 0.0, None, AL.is_gt)
    nc.vector.copy_predicated(s.osl2((0, 1, 1), (2, 0, 0)), mr[:], cnhm[:])

    # ---------------- store ----------------
    if last:
        p0, pn, row0 = 102, 24, 744
    else:
        p0, pn, row0 = 2, 124, r0
    src3 = out3[p0: p0 + pn, :].rearrange("p (c w) -> p c w", c=3)
    nc.gpsimd.dma_start(out_r[row0: row0 + pn, :, :], src3)


def build_nc():
    nc = bacc.Bacc("TRN2", target_bir_lowering=False, debug=False, num_devices=NCORES)
    x_in = nc.declare_dram_parameter("x", [4, H, H], F, isOutput=False)
    bands_in = nc.declare_dram_parameter("bands", [128, len(BAND_ORDER) * 128], H16, isOutput=False)
    out = nc.declare_dram_parameter("out", [3, 2 * H, 2 * H], F, isOutput=True)
    out_r = out[:].rearrange("c (r two) w -> r c (two w)", two=2)

    with tile.TileContext(nc) as tc, ExitStack() as ctx:
        cst = ctx.enter_context(tc.tile_pool(name="consts", bufs=1))
        pl = ctx.enter_context(tc.tile_pool(name="planes", bufs=2))
        gr = ctx.enter_context(tc.tile_pool(name="greens", bufs=3))
        tmp = ctx.enter_context(tc.tile_pool(name="temps", bufs=32))
        outp = ctx.enter_context(tc.tile_pool(name="outs", bufs=2))
        psp = ctx.enter_context(tc.tile_pool(name="ps", bufs=2, space=bass.MemorySpace.PSUM))
        ez = ctx.enter_context(tc.tile_pool(name="ez", bufs=2))
        bands = cst.tile([128, len(BAND_ORDER) * 128], H16, tag="bands", name="bands")
        nc.sync.dma_start(bands[:], bands_in[:])
        zt = cst.tile([128, SW], H16, tag="zt", name="zt")
        nc.vector.memset(zt[:], 0.0)
        B = lambda n: bands[:, BAND_ORDER.index(n) * 128: BAND_ORDER.index(n) * 128 + 128]
        order = list(range(len(STRIPS)))
        strips = []
        for si in order:
            s = S(nc, (pl, gr, tmp, outp, psp, ez), STRIPS[si], si == 0,
                  si == len(STRIPS) - 1)
            s.zt = zt
            strips.append(s)
        for i, s in enumerate(strips):
            _load_strip(nc, s, x_in[:])
            _build_strip(nc, s, x_in[:], out_r, B)
    nc.compile()
    return nc


_NC_CACHE = None


def kernel(x: np.ndarray) -> np.ndarray:
    global _NC_CACHE
    if _NC_CACHE is None:
        _NC_CACHE = build_nc()
    x = np.ascontiguousarray(x, dtype=np.float32)
    bnp = _bands_np()
    in_maps = [{"x": x[i], "bands": bnp} for i in range(NCORES)]
    res = run_bass_kernel_spmd(_NC_CACHE, in_maps, list(range(NCORES)))
    return np.stack([res.results[i]["out"] for i in range(NCORES)], axis=0)
